# revision 16
# baseline (speedup 1.0000x reference)
"""Trainium2 Bass kernel for nn_DecoderWithPositionLayer (8 NeuronCores, SPMD).

Sharding: core c -> (batch b=c//2, row-half p=c%2); zero collectives.
Each pair of cores shares one batch element: kk/vv (self-attn keys/values,
from block0) and block3 (cross-attn keys/values) are computed for all 1024
rows on both cores; queries / MLPs / outputs are split 512 rows per core.

The 512 owned query rows are interleaved 128-row tiles (half p owns original
row-tiles {2k+p}).  The host permutes the q-token axis per core so permuted
tile 2m = own tile m, permuted tile 2m+1 = the other core's tile m.  With
that, "owned tokens" are the even permuted 128-tiles on every core and the
causal-attention schedule is identical across cores (SPMD: one program).

Device layout is feature-major: activations are x^T [feature-on-partition,
token-on-free], so every Dense matmul uses the natural [d_in, d_out] weight
as the stationary lhsT and no activation transposes exist anywhere.

LayerNorm (gamma=1, beta=0, dense biases=0 for this problem):
  block(x) = rstd[i] * relu((x - mu[i]) @ w1) @ w2
  - stats via ones-matmuls over the partition axis
  - mean-shift folded into Dense1 as a K=1 augmented matmul
    (lhsT = colsum(w1), rhs = -mu)
  - rstd applied once on the relu output (relu commutes with positive scale)
  - rstd = exp(-0.5*ln(var+eps)) -> only ln/exp ACT tables needed

Self-attn scores are built transposed [j, i] (keys on partitions): softmax
denominator = ones-matmul over partitions, A@V consumes att directly (no
transposes), and the 3-entry position-bias gather becomes two per-head
diagonal matmuls into the scores PSUM:
  bias = d0[i] + d1[i]*[pos>=1] + d2[i]*[pos>=2]  (d0 drops: softmax shift
  invariance), term_r = (mask_r tile as lhsT) @ diag(delta_r_h[i]).
Causal mask = (-30)*[j > i] via one more diag-matmul; scores are bounded
(|s| < ~9) so exp needs no max subtraction.  1/sum via exp(-ln(sum)).
"""

import os
import sys

for _p in ("/opt/trn_rl_repo",):
    if os.path.isdir(_p) and _p not in sys.path:
        sys.path.insert(0, _p)

import numpy as np
import ml_dtypes

import concourse.bass as bass
import concourse.mybir as mybir
import concourse.tile as tile
from concourse import bacc
from concourse.masks import make_identity

BF = mybir.dt.bfloat16
F32 = mybir.dt.float32
bf16 = ml_dtypes.bfloat16

B, N, D, H, DH = 4, 1024, 1024, 8, 128
EPS = 1e-3
NT = N // 128            # 8 token tiles
NOWN = N // 2            # 512 owned query rows per core
KT = 4                   # owned row-tiles per core
INV_SQRT_DH = float(1.0 / np.sqrt(np.float32(DH)))
LN128 = float(np.log(128.0))
KILL = -30.0
A = mybir.AluOpType
AF = mybir.ActivationFunctionType

# kmin[jt] = first live packed query slot for (permuted) key tile jt
KMIN = [0, 0, 1, 1, 2, 2, 3, 3]

ts = bass.ts


def build_program():
    nc = bacc.Bacc(None)
    dp = nc.declare_dram_parameter
    qT_d = dp("qT", [D, N], BF, isOutput=False)
    vT_d = dp("vT", [D, N], BF, isOutput=False)
    w1_d = [dp(f"w{i}_1", [1024, 512], BF, isOutput=False) for i in range(5)]
    w2_dims = [3072, 1024, 1024, 2048, 1024]
    w2_d = [dp(f"w{i}_2", [512, w2_dims[i]], BF, isOutput=False) for i in range(5)]
    csum_d = dp("csum", [1, 5 * 512], BF, isOutput=False)
    pd_d = dp("pos_delta", [DH, 2 * H], BF, isOutput=False)
    m1_d = dp("m1", [NOWN, N], BF, isOutput=False)
    m2_d = dp("m2", [NOWN, N], BF, isOutput=False)
    m3_d = dp("m3", [128, NT, 128], BF, isOutput=False)
    out_d = dp("out", [D, NOWN], F32, isOutput=True)

    def mm(out, lhsT, rhs, start, stop, skip=False):
        nc.tensor.matmul(out, lhsT, rhs, start=start, stop=stop,
                         skip_group_check=skip)

    with tile.TileContext(nc) as tc:
        from contextlib import ExitStack
        with ExitStack() as ctx:
            consts = ctx.enter_context(tc.tile_pool(name="consts", bufs=1))
            persist = ctx.enter_context(tc.tile_pool(name="persist", bufs=1))
            work = ctx.enter_context(tc.tile_pool(name="work", bufs=4))
            small = ctx.enter_context(tc.tile_pool(name="small", bufs=2))
            ps_mm = ctx.enter_context(tc.tile_pool(name="ps_mm", bufs=4, space="PSUM"))
            ps_y = ctx.enter_context(tc.tile_pool(name="ps_y", bufs=2, space="PSUM"))
            ps_sm = ctx.enter_context(tc.tile_pool(name="ps_sm", bufs=2, space="PSUM"))

            # ---- constants ----
            ident = consts.tile([128, 128], BF)
            make_identity(nc, ident)
            diag_kill = consts.tile([128, 128], BF)
            nc.vector.tensor_scalar_mul(diag_kill, ident, KILL)
            ones_col = consts.tile([128, 1], BF)
            nc.vector.memset(ones_col, 1.0)
            ones_row = consts.tile([1, 128], BF)
            nc.vector.memset(ones_row, 1.0)
            neg_invN = consts.tile([128, 1], BF)
            nc.vector.memset(neg_invN, -1.0 / D)
            pos_invN = consts.tile([128, 1], BF)
            nc.vector.memset(pos_invN, 1.0 / D)
            eps_t = consts.tile([1, 1], F32)
            nc.vector.memset(eps_t, EPS)
            nlb_t = consts.tile([1, 1], F32)
            nc.vector.memset(nlb_t, -0.5 * LN128)
            cs_sb = consts.tile([1, 5 * 512], BF)
            nc.sync.dma_start(cs_sb, csum_d[:, :])
            pd_sb = consts.tile([DH, 2 * H], BF)
            nc.sync.dma_start(pd_sb, pd_d[:, :])

            # ---- persistent activations ----
            kkT = persist.tile([128, H, N], BF, tag="kkT")   # [dh, h, j]
            vv = persist.tile([128, NT, D], BF, tag="vv")    # [j%128, jt, d]
            qqT = persist.tile([128, H, NOWN], BF, tag="qqT")  # x 1/sqrt(dh)
            k2T = persist.tile([128, H, N], BF, tag="kkT")
            v2 = persist.tile([128, NT, D], BF, tag="vv")
            q2T = persist.tile([128, H, NOWN], BF, tag="qqT")  # x 1/sqrt(dh)
            yT = persist.tile([128, H, NOWN], BF, tag="yT")
            y2T = persist.tile([128, H, NOWN], BF, tag="yT")
            q1 = persist.tile([128, H, NOWN], BF)
            delta = persist.tile([128, H, KT, 2], F32)

            def own_ap(t, f):
                """[128, F, 1024] tile -> [128, 4, 128] strided AP (even 128-tiles)."""
                return t[:, f, :].rearrange("p (a two c) -> p a two c",
                                            two=2, c=128)[:, :, 0, :]

            # ================= helpers =================
            def stats(get_tile, n_blocks, qq_scale=False):
                negmu = small.tile([1, n_blocks, 512], BF, tag="negmu", bufs=2)
                rstd = small.tile([1, n_blocks, 512], BF, tag="rstd", bufs=2)
                rstd_s = None
                if qq_scale:
                    rstd_s = small.tile([1, n_blocks, 512], BF, tag="rstd_s",
                                        bufs=2, name="rstd_s")
                for blk in range(n_blocks):
                    ps_nm = ps_sm.tile([1, 512], F32, tag="sm")
                    ps_ms = ps_sm.tile([1, 512], F32, tag="sm")
                    for f in range(8):
                        x = get_tile(f, blk)
                        mm(ps_nm, neg_invN, x, start=(f == 0), stop=(f == 7))
                        sq = work.tile([128, 512], BF, tag="sq", bufs=2)
                        nc.scalar.activation(sq, x, AF.Square)
                        mm(ps_ms, pos_invN, sq, start=(f == 0), stop=(f == 7))
                    nc.scalar.copy(negmu[:, blk, :], ps_nm)
                    musq = small.tile([1, 512], F32, tag="musq")
                    nc.scalar.activation(musq, negmu[:, blk, :], AF.Square)
                    var = small.tile([1, 512], F32, tag="var")
                    nc.vector.tensor_tensor(var, ps_ms, musq, A.subtract)
                    lnv = small.tile([1, 512], F32, tag="lnv")
                    nc.scalar.activation(lnv, var, AF.Ln, bias=eps_t)
                    nc.scalar.activation(rstd[:, blk, :], lnv, AF.Exp, scale=-0.5)
                    if qq_scale:
                        nc.scalar.activation(rstd_s[:, blk, :], lnv, AF.Exp,
                                             scale=-0.5, bias=nlb_t)
                return negmu, rstd, rstd_s

            def rstd_bcast(rstd_row):
                """[1, 512] bf16 -> [128, 512] bf16 (broadcast down partitions)."""
                psb = ps_mm.tile([128, 512], F32, tag="mm")
                mm(psb, ones_row, rstd_row, start=True, stop=True)
                rb = work.tile([128, 512], BF, tag="rb", bufs=2)
                nc.vector.tensor_copy(rb, psb)
                return rb

            def load_w1(i):
                t = work.tile([128, 8, 512], BF, tag="w1", bufs=1)
                nc.sync.dma_start(
                    t, w1_d[i].rearrange("(k p) m -> p k m", p=128))
                return t

            def load_w2(i, col0):
                t = work.tile([128, 4, 1024], BF, tag="w2", bufs=2, name="w2c")
                nc.sync.dma_start(
                    t, w2_d[i][:, col0:col0 + 1024].rearrange(
                        "(k p) m -> p k m", p=128))
                return t

            def dense1(get_tile, n_blocks, w1_sb, blk_idx, negmu, rstd):
                """relu((x-mu)@w1)*rstd -> [128, 4, n_blocks*512] bf16."""
                r = work.tile([128, 4, n_blocks * 512], BF, tag="relu", bufs=1,
                              padded_shape=[128, 4, 1024])
                for blk in range(n_blocks):
                    rb = rstd_bcast(rstd[:, blk, :])
                    for mt in range(4):
                        ps = ps_mm.tile([128, 512], F32, tag="mm")
                        for kt in range(8):
                            mm(ps, w1_sb[:, kt, ts(mt, 128)], get_tile(kt, blk),
                               start=(kt == 0), stop=False)
                        mm(ps, cs_sb[:, blk_idx * 512 + mt * 128:
                                     blk_idx * 512 + (mt + 1) * 128],
                           negmu[:, blk, :], start=False, stop=True)
                        rt = work.tile([128, 512], BF, tag="rt", bufs=2)
                        nc.scalar.activation(rt, ps, AF.Relu)
                        nc.vector.tensor_tensor(
                            r[:, mt, blk * 512:(blk + 1) * 512], rt, rb, A.mult)
                return r

            # ================= block0 (on q) =================
            qTf = work.tile([128, 8, N], BF, tag="xTf", bufs=1,
                            padded_shape=[128, 8, N])
            nc.sync.dma_start(qTf, qT_d.rearrange("(f p) n -> p f n", p=128))

            negmu_q, rstd_q, _ = stats(
                lambda f, blk: qTf[:, f, blk * 512:(blk + 1) * 512], 2)
            w0_1 = load_w1(0)
            relu0 = dense1(lambda kt, blk: qTf[:, kt, blk * 512:(blk + 1) * 512],
                           2, w0_1, 0, negmu_q, rstd_q)
            # kk: feature-major, act cols [1024:2048)
            w0_2k = load_w2(0, 1024)
            for blk in range(2):
                for ft in range(8):
                    ps = ps_mm.tile([128, 512], F32, tag="mm")
                    for kt in range(4):
                        mm(ps, w0_2k[:, kt, ft * 128:(ft + 1) * 128],
                           relu0[:, kt, blk * 512:(blk + 1) * 512],
                           start=(kt == 0), stop=(kt == 3))
                    nc.vector.tensor_copy(kkT[:, ft, blk * 512:(blk + 1) * 512], ps)
            # vv: token-major, act cols [2048:3072)
            w0_2v = load_w2(0, 2048)
            for jt in range(NT):
                for dh2 in range(2):
                    ps = ps_mm.tile([128, 512], F32, tag="mm")
                    for kt in range(4):
                        mm(ps, relu0[:, kt, jt * 128:(jt + 1) * 128],
                           w0_2v[:, kt, dh2 * 512:(dh2 + 1) * 512],
                           start=(kt == 0), stop=(kt == 3))
                    nc.scalar.copy(vv[:, jt, dh2 * 512:(dh2 + 1) * 512], ps)
            # qq: feature-major, act cols [0:1024), owned tokens, x 1/sqrt(dh)
            # (relu0 was scaled by rstd; qq needs rstd/sqrt(dh) -> extra factor here)
            w0_2q = load_w2(0, 0)
            for ft in range(8):
                ps = ps_mm.tile([128, 512], F32, tag="mm")
                for kt in range(4):
                    mm(ps, w0_2q[:, kt, ft * 128:(ft + 1) * 128],
                       own_ap(relu0, kt), start=(kt == 0), stop=(kt == 3))
                nc.scalar.mul(qqT[:, ft, :], ps, INV_SQRT_DH)

            # dots -> per-head bias deltas (kk at owned rows . pos_delta)
            for h in range(H):
                for k in range(KT):
                    psd = ps_sm.tile([128, 2], F32, tag="sm")
                    mm(psd, kkT[:, h, 256 * k:256 * k + 128],
                       pd_sb[:, 2 * h:2 * h + 2], start=True, stop=True)
                    nc.vector.tensor_copy(delta[:, h, k, :], psd)

            # ================= self-attention =================
            mk1 = work.tile([128, KT, N], BF, tag="mk1", bufs=1)
            mk2 = work.tile([128, KT, N], BF, tag="mk2", bufs=1)
            mk3 = work.tile([128, NT, 128], BF, tag="mk3", bufs=1)
            nc.sync.dma_start(mk1, m1_d.rearrange("(k p) n -> p k n", p=128))
            nc.sync.dma_start(mk2, m2_d.rearrange("(k p) n -> p k n", p=128))
            nc.sync.dma_start(mk3[:, :, :], m3_d[:, :, :])

            for h in range(H):
                dg1 = work.tile([128, KT, 128], BF, tag="dg1", bufs=2)
                dg2 = work.tile([128, KT, 128], BF, tag="dg2", bufs=2)
                for k in range(KT):
                    nc.vector.tensor_scalar_mul(dg1[:, k, :], ident,
                                                delta[:, h, k, 0:1])
                    nc.vector.tensor_scalar_mul(dg2[:, k, :], ident,
                                                delta[:, h, k, 1:2])
                att = work.tile([128, NT, NOWN], BF, tag="att", bufs=2)
                ps_yh = ps_y.tile([128, NOWN], F32, tag="y")
                ps_sum = ps_sm.tile([1, NOWN], F32, tag="sm")
                for jt in range(NT):
                    ilo = KMIN[jt] * 128
                    ps_s = ps_mm.tile([128, NOWN], F32, tag="mm")
                    mm(ps_s[:, ilo:], kkT[:, h, jt * 128:(jt + 1) * 128],
                       qqT[:, h, ilo:], start=True, stop=False)
                    for k in range(KMIN[jt], KT):
                        mm(ps_s[:, k * 128:(k + 1) * 128],
                           mk1[:, k, jt * 128:(jt + 1) * 128], dg1[:, k, :],
                           start=False, stop=False)
                        mm(ps_s[:, k * 128:(k + 1) * 128],
                           mk2[:, k, jt * 128:(jt + 1) * 128], dg2[:, k, :],
                           start=False, stop=False)
                    kd = jt // 2
                    mm(ps_s[:, kd * 128:(kd + 1) * 128],
                       mk3[:, jt, :], diag_kill, start=False, stop=True)
                    nc.scalar.activation(att[:, jt, ilo:], ps_s[:, ilo:], AF.Exp)
                    mm(ps_sum[:, ilo:], ones_col, att[:, jt, ilo:],
                       start=(jt == 0), stop=(jt == NT - 1))
                    mm(ps_yh[:, ilo:], vv[:, jt, h * 128:(h + 1) * 128],
                       att[:, jt, ilo:], start=(jt == 0), stop=(jt == NT - 1))
                lnsum = small.tile([1, NOWN], F32, tag="lnsum")
                nc.scalar.activation(lnsum, ps_sum, AF.Ln)
                rec = small.tile([1, NOWN], BF, tag="rec")
                nc.scalar.activation(rec, lnsum, AF.Exp, scale=-1.0)
                rb = rstd_bcast(rec)
                nc.vector.tensor_tensor(yT[:, h, :], ps_yh, rb, A.mult)

            # ================= block1 + residual =================
            negmu_y, rstd_y, _ = stats(lambda f, blk: yT[:, f, :], 1)
            w1_1 = load_w1(1)
            relu1 = dense1(lambda kt, blk: yT[:, kt, :], 1, w1_1, 1,
                           negmu_y, rstd_y)
            w1_2 = load_w2(1, 0)
            for ft in range(8):
                ps = ps_mm.tile([128, 512], F32, tag="mm")
                for kt in range(4):
                    mm(ps, w1_2[:, kt, ft * 128:(ft + 1) * 128],
                       relu1[:, kt, :], start=(kt == 0), stop=(kt == 3))
                nc.vector.tensor_tensor(
                    q1[:, ft, :].rearrange("p (a c) -> p a c", c=128),
                    ps.rearrange("p (a c) -> p a c", c=128),
                    own_ap(qTf, ft), A.add)

            # ================= block2 -> q2 =================
            negmu_q1, rstd_q1, _ = stats(lambda f, blk: q1[:, f, :], 1)
            w2_1 = load_w1(2)
            relu2 = dense1(lambda kt, blk: q1[:, kt, :], 1, w2_1, 2,
                           negmu_q1, rstd_q1)
            w2_2 = load_w2(2, 0)
            for ft in range(8):
                ps = ps_mm.tile([128, 512], F32, tag="mm")
                for kt in range(4):
                    mm(ps, w2_2[:, kt, ft * 128:(ft + 1) * 128],
                       relu2[:, kt, :], start=(kt == 0), stop=(kt == 3))
                nc.scalar.mul(q2T[:, ft, :], ps, INV_SQRT_DH)

            # ================= block3 (on v_enc) -> k2, v2 =================
            vTf = work.tile([128, 8, N], BF, tag="xTf", bufs=1,
                            padded_shape=[128, 8, N])
            nc.sync.dma_start(vTf, vT_d.rearrange("(f p) n -> p f n", p=128))
            negmu_v, rstd_v, _ = stats(
                lambda f, blk: vTf[:, f, blk * 512:(blk + 1) * 512], 2)
            w3_1 = load_w1(3)
            relu3 = dense1(lambda kt, blk: vTf[:, kt, blk * 512:(blk + 1) * 512],
                           2, w3_1, 3, negmu_v, rstd_v)
            w3_2 = load_w2(3, 0)
            # k2: feature-major, cols [0:1024)
            for blk in range(2):
                for ft in range(8):
                    ps = ps_mm.tile([128, 512], F32, tag="mm")
                    for kt in range(4):
                        mm(ps, w3_2[:, kt, ft * 128:(ft + 1) * 128],
                           relu3[:, kt, blk * 512:(blk + 1) * 512],
                           start=(kt == 0), stop=(kt == 3))
                    nc.vector.tensor_copy(k2T[:, ft, blk * 512:(blk + 1) * 512], ps)
            # v2: token-major, cols [1024:2048)
            w3_2v = load_w2(3, 1024)
            for jt in range(NT):
                for dh2 in range(2):
                    ps = ps_mm.tile([128, 512], F32, tag="mm")
                    for kt in range(4):
                        mm(ps, relu3[:, kt, jt * 128:(jt + 1) * 128],
                           w3_2v[:, kt, dh2 * 512:(dh2 + 1) * 512],
                           start=(kt == 0), stop=(kt == 3))
                    nc.scalar.copy(v2[:, jt, dh2 * 512:(dh2 + 1) * 512], ps)

            # ================= cross-attention =================
            for h in range(H):
                attx = work.tile([128, NT, NOWN], BF, tag="att", bufs=2)
                ps_yh = ps_y.tile([128, NOWN], F32, tag="y")
                ps_sum = ps_sm.tile([1, NOWN], F32, tag="sm")
                for jt in range(NT):
                    ps_s = ps_mm.tile([128, NOWN], F32, tag="mm")
                    mm(ps_s, k2T[:, h, jt * 128:(jt + 1) * 128], q2T[:, h, :],
                       start=True, stop=True)
                    nc.scalar.activation(attx[:, jt, :], ps_s, AF.Exp)
                    mm(ps_sum, ones_col, attx[:, jt, :],
                       start=(jt == 0), stop=(jt == NT - 1))
                    mm(ps_yh, v2[:, jt, h * 128:(h + 1) * 128], attx[:, jt, :],
                       start=(jt == 0), stop=(jt == NT - 1))
                lnsum = small.tile([1, NOWN], F32, tag="lnsum")
                nc.scalar.activation(lnsum, ps_sum, AF.Ln)
                rec = small.tile([1, NOWN], BF, tag="rec")
                nc.scalar.activation(rec, lnsum, AF.Exp, scale=-1.0)
                rb = rstd_bcast(rec)
                nc.vector.tensor_tensor(y2T[:, h, :], ps_yh, rb, A.mult)

            # ================= block4 + residual -> out =================
            negmu_y2, rstd_y2, _ = stats(lambda f, blk: y2T[:, f, :], 1)
            w4_1 = load_w1(4)
            relu4 = dense1(lambda kt, blk: y2T[:, kt, :], 1, w4_1, 4,
                           negmu_y2, rstd_y2)
            w4_2 = load_w2(4, 0)
            for ft in range(8):
                ps = ps_mm.tile([128, 512], F32, tag="mm")
                for kt in range(4):
                    mm(ps, w4_2[:, kt, ft * 128:(ft + 1) * 128],
                       relu4[:, kt, :], start=(kt == 0), stop=(kt == 3))
                out_t = work.tile([128, NOWN], F32, tag="out_t", bufs=2,
                                  name="out_t")
                nc.vector.tensor_tensor(out_t, ps, q1[:, ft, :], A.add)
                nc.sync.dma_start(out_d[ft * 128:(ft + 1) * 128, :], out_t)
    if not nc.is_finalized():
        nc.finalize()
    return nc


_NC = None
TRACE = False
LAST_EXEC_NS = None


def _get_program():
    global _NC
    if _NC is None:
        _NC = build_program()
    return _NC


def _perm_tiles(p):
    return [t for m in range(4) for t in (2 * m + p, 2 * m + 1 - p)]


def _host_prep(inputs):
    """Build the 8 per-core input maps (all host-side, untimed)."""
    f32 = np.float32
    q = np.asarray(inputs["q"], f32)
    v_enc = np.asarray(inputs["v_enc"], f32)
    positions = np.asarray(inputs["positions"])
    pos_table = np.asarray(inputs["pos_table"], f32)

    shared = {}
    for i in range(5):
        w1 = np.asarray(inputs[f"b{i}_w1"], f32)
        w2 = np.asarray(inputs[f"b{i}_w2"], f32)
        shared[f"w{i}_1"] = np.ascontiguousarray(w1).astype(bf16)
        shared[f"w{i}_2"] = np.ascontiguousarray(w2).astype(bf16)
    shared["csum"] = np.concatenate(
        [np.asarray(inputs[f"b{i}_w1"], f32).sum(0) for i in range(5)]
    ).reshape(1, 5 * 512).astype(bf16)
    pt = pos_table.reshape(3, H, DH)
    pd = np.empty((DH, 2 * H), f32)
    for h in range(H):
        pd[:, 2 * h] = pt[1, h] - pt[0, h]
        pd[:, 2 * h + 1] = pt[2, h] - pt[1, h]
    shared["pos_delta"] = pd.astype(bf16)

    in_maps = []
    metas = []
    for c in range(8):
        b, p = c // 2, c % 2
        ptiles = _perm_tiles(p)
        perm = np.concatenate([np.arange(128 * t, 128 * t + 128) for t in ptiles])
        own = perm.reshape(8, 128)[0::2].reshape(-1)   # packed owned rows (orig idx)
        m = dict(shared)
        m["qT"] = np.ascontiguousarray(q[b].T[:, perm]).astype(bf16)
        m["vT"] = np.ascontiguousarray(v_enc[b].T).astype(bf16)
        pos = positions[b]
        po = pos[own][:, perm]
        m["m1"] = (po >= 1).astype(bf16)
        m["m2"] = (po >= 2).astype(bf16)
        m3 = np.zeros((128, NT, 128), f32)
        for jt in range(NT):
            kd = jt // 2
            oi = own[kd * 128:(kd + 1) * 128]
            oj = perm[jt * 128:(jt + 1) * 128]
            m3[:, jt, :] = (oj[None, :] > oi[:, None])
        m["m3"] = m3.astype(bf16)
        in_maps.append(m)
        metas.append((b, own))
    return in_maps, metas


def _install_profile_shim():
    """Provide antenv.axon_hooks (missing in this image) so bass_utils can
    NTFF-profile under axon.  Mirrors trn_boot._ntff_profile_via_ctypes."""
    import sys as _sys
    if "antenv.axon_hooks" in _sys.modules:
        return
    import types
    import ctypes
    import contextlib

    mod = types.ModuleType("antenv.axon_hooks")
    _hook_holder = {"hook": None}

    def set_axon_ntff_profile_hook(h):
        _hook_holder["hook"] = h

    def get_axon_ntff_profile_hook():
        return _hook_holder["hook"]

    mod.set_axon_ntff_profile_hook = set_axon_ntff_profile_hook
    mod.get_axon_ntff_profile_hook = get_axon_ntff_profile_hook
    _sys.modules["antenv.axon_hooks"] = mod

    so_path = "/opt/axon/libaxon_pjrt.so"
    if not os.path.exists(so_path):
        return
    lib = ctypes.CDLL(so_path)
    if not hasattr(lib, "axon_start_nrt_profile"):
        return
    lib.axon_start_nrt_profile.argtypes = [
        ctypes.POINTER(ctypes.c_int64), ctypes.c_size_t]
    lib.axon_start_nrt_profile.restype = ctypes.c_int64
    lib.axon_stop_nrt_profile.argtypes = [ctypes.c_char_p]
    lib.axon_stop_nrt_profile.restype = ctypes.c_int64

    @contextlib.contextmanager
    def _hook(output_dir, device_ids):
        import jax
        jax.devices()
        if device_ids:
            ids = (ctypes.c_int64 * len(device_ids))(*device_ids)
            rc = lib.axon_start_nrt_profile(ids, len(device_ids))
        else:
            rc = lib.axon_start_nrt_profile(None, 0)
        if rc != 0:
            raise RuntimeError(f"axon_start_nrt_profile rc={rc}")
        try:
            yield
        finally:
            n = lib.axon_stop_nrt_profile(str(output_dir).encode())
            print(f"profile: {n} file(s) written to {output_dir}")

    set_axon_ntff_profile_hook(_hook)


def kernel(**inputs):
    global LAST_EXEC_NS
    if TRACE:
        _install_profile_shim()
    from concourse.bass_utils import run_bass_kernel_spmd
    nc = _get_program()
    in_maps, metas = _host_prep(inputs)
    res = run_bass_kernel_spmd(nc, in_maps, core_ids=list(range(8)),
                               trace=TRACE)
    LAST_EXEC_NS = res.exec_time_ns
    out = np.zeros((B, N, D), np.float32)
    for c in range(8):
        b, own = metas[c]
        out[b, own, :] = res.results[c]["out"].T
    return out


if __name__ == "__main__":
    build_program()
    print("build ok")


# revision 17
# speedup vs baseline: 1.0291x; 1.0291x over previous
"""Trainium2 Bass kernel for nn_DecoderWithPositionLayer (8 NeuronCores, SPMD).

Sharding: core c -> (batch b=c//2, row-half p=c%2); zero collectives.
Each pair of cores shares one batch element: kk/vv (self-attn keys/values,
from block0) and block3 (cross-attn keys/values) are computed for all 1024
rows on both cores; queries / MLPs / outputs are split 512 rows per core.

The 512 owned query rows are interleaved 128-row tiles (half p owns original
row-tiles {2k+p}).  The host permutes the q-token axis per core so permuted
tile 2m = own tile m, permuted tile 2m+1 = the other core's tile m.  With
that, "owned tokens" are the even permuted 128-tiles on every core and the
causal-attention schedule is identical across cores (SPMD: one program).

Device layout is feature-major: activations are x^T [feature-on-partition,
token-on-free], so every Dense matmul uses the natural [d_in, d_out] weight
as the stationary lhsT and no activation transposes exist anywhere.

LayerNorm (gamma=1, beta=0, dense biases=0 for this problem):
  block(x) = rstd[i] * relu((x - mu[i]) @ w1) @ w2
  - stats via ones-matmuls over the partition axis
  - mean-shift folded into Dense1 as a K=1 augmented matmul
    (lhsT = colsum(w1), rhs = -mu)
  - rstd applied once on the relu output (relu commutes with positive scale)
  - rstd = exp(-0.5*ln(var+eps)) -> only ln/exp ACT tables needed

Self-attn scores are built transposed [j, i] (keys on partitions): softmax
denominator = ones-matmul over partitions, A@V consumes att directly (no
transposes), and the 3-entry position-bias gather becomes two per-head
diagonal matmuls into the scores PSUM:
  bias = d0[i] + d1[i]*[pos>=1] + d2[i]*[pos>=2]  (d0 drops: softmax shift
  invariance), term_r = (mask_r tile as lhsT) @ diag(delta_r_h[i]).
Causal mask = (-30)*[j > i] via one more diag-matmul; scores are bounded
(|s| < ~9) so exp needs no max subtraction.  1/sum via exp(-ln(sum)).
"""

import os
import sys

for _p in ("/opt/trn_rl_repo",):
    if os.path.isdir(_p) and _p not in sys.path:
        sys.path.insert(0, _p)

import numpy as np
import ml_dtypes

import concourse.bass as bass
import concourse.mybir as mybir
import concourse.tile as tile
from concourse import bacc
from concourse.masks import make_identity

BF = mybir.dt.bfloat16
F32 = mybir.dt.float32
bf16 = ml_dtypes.bfloat16

B, N, D, H, DH = 4, 1024, 1024, 8, 128
EPS = 1e-3
NT = N // 128            # 8 token tiles
NOWN = N // 2            # 512 owned query rows per core
KT = 4                   # owned row-tiles per core
INV_SQRT_DH = float(1.0 / np.sqrt(np.float32(DH)))
LN128 = float(np.log(128.0))
KILL = -30.0
A = mybir.AluOpType
AF = mybir.ActivationFunctionType

# kmin[jt] = first live packed query slot for (permuted) key tile jt
KMIN = [0, 0, 1, 1, 2, 2, 3, 3]

ts = bass.ts


def build_program():
    nc = bacc.Bacc(None)
    dp = nc.declare_dram_parameter
    qT_d = dp("qT", [D, N], BF, isOutput=False)
    vT_d = dp("vT", [D, N], BF, isOutput=False)
    w1_d = [dp(f"w{i}_1", [1024, 512], BF, isOutput=False) for i in range(5)]
    w2_dims = [3072, 1024, 1024, 2048, 1024]
    w2_d = [dp(f"w{i}_2", [512, w2_dims[i]], BF, isOutput=False) for i in range(5)]
    csum_d = dp("csum", [1, 5 * 512], BF, isOutput=False)
    pd_d = dp("pos_delta", [DH, 2 * H], BF, isOutput=False)
    m1_d = dp("m1", [NOWN, N], BF, isOutput=False)
    m2_d = dp("m2", [NOWN, N], BF, isOutput=False)
    m3_d = dp("m3", [128, NT, 128], BF, isOutput=False)
    out_d = dp("out", [D, NOWN], F32, isOutput=True)

    def mm(out, lhsT, rhs, start, stop, skip=False):
        nc.tensor.matmul(out, lhsT, rhs, start=start, stop=stop,
                         skip_group_check=skip)

    with tile.TileContext(nc) as tc:
        from contextlib import ExitStack
        with ExitStack() as ctx:
            consts = ctx.enter_context(tc.tile_pool(name="consts", bufs=1))
            persist = ctx.enter_context(tc.tile_pool(name="persist", bufs=1))
            work = ctx.enter_context(tc.tile_pool(name="work", bufs=4))
            small = ctx.enter_context(tc.tile_pool(name="small", bufs=2))
            ps_mm = ctx.enter_context(tc.tile_pool(name="ps_mm", bufs=4, space="PSUM"))
            ps_y = ctx.enter_context(tc.tile_pool(name="ps_y", bufs=2, space="PSUM"))
            ps_sm = ctx.enter_context(tc.tile_pool(name="ps_sm", bufs=2, space="PSUM"))

            # ---- constants ----
            ident = consts.tile([128, 128], BF)
            make_identity(nc, ident)
            diag_kill = consts.tile([128, 128], BF)
            nc.vector.tensor_scalar_mul(diag_kill, ident, KILL)
            ones_col = consts.tile([128, 1], BF)
            nc.vector.memset(ones_col, 1.0)
            ones_row = consts.tile([1, 128], BF)
            nc.vector.memset(ones_row, 1.0)
            neg_invN = consts.tile([128, 1], BF)
            nc.vector.memset(neg_invN, -1.0 / D)
            pos_invN = consts.tile([128, 1], BF)
            nc.vector.memset(pos_invN, 1.0 / D)
            eps_t = consts.tile([1, 1], F32)
            nc.vector.memset(eps_t, EPS)
            nlb_t = consts.tile([1, 1], F32)
            nc.vector.memset(nlb_t, -0.5 * LN128)
            cs_sb = consts.tile([1, 5 * 512], BF)
            nc.sync.dma_start(cs_sb, csum_d[:, :])
            pd_sb = consts.tile([DH, 2 * H], BF)
            nc.sync.dma_start(pd_sb, pd_d[:, :])

            # ---- persistent activations ----
            kkT = persist.tile([128, H, N], BF, tag="kkT")   # [dh, h, j]
            vv = persist.tile([128, NT, D], BF, tag="vv")    # [j%128, jt, d]
            qqT = persist.tile([128, H, NOWN], BF, tag="qqT")  # x 1/sqrt(dh)
            k2T = persist.tile([128, H, N], BF, tag="kkT")
            v2 = persist.tile([128, NT, D], BF, tag="vv")
            q2T = persist.tile([128, H, NOWN], BF, tag="qqT")  # x 1/sqrt(dh)
            yT = persist.tile([128, H, NOWN], BF, tag="yT")
            y2T = persist.tile([128, H, NOWN], BF, tag="yT")
            q1 = persist.tile([128, H, NOWN], BF)
            delta = persist.tile([128, H, KT, 2], F32)

            def own_ap(t, f):
                """[128, F, 1024] tile -> [128, 4, 128] strided AP (even 128-tiles)."""
                return t[:, f, :].rearrange("p (a two c) -> p a two c",
                                            two=2, c=128)[:, :, 0, :]

            # ================= helpers =================
            def stats(get_tile, n_blocks, qq_scale=False):
                negmu = small.tile([1, n_blocks, 512], BF, tag="negmu", bufs=2)
                rstd = small.tile([1, n_blocks, 512], BF, tag="rstd", bufs=2)
                rstd_s = None
                if qq_scale:
                    rstd_s = small.tile([1, n_blocks, 512], BF, tag="rstd_s",
                                        bufs=2, name="rstd_s")
                for blk in range(n_blocks):
                    ps_nm = ps_sm.tile([1, 512], F32, tag="sm")
                    ps_ms = ps_sm.tile([1, 512], F32, tag="sm")
                    for f in range(8):
                        x = get_tile(f, blk)
                        mm(ps_nm, neg_invN, x, start=(f == 0), stop=(f == 7))
                        sq = work.tile([128, 512], BF, tag="sq", bufs=2)
                        nc.vector.tensor_tensor(sq, x, x, A.mult)
                        mm(ps_ms, pos_invN, sq, start=(f == 0), stop=(f == 7))
                    nc.scalar.copy(negmu[:, blk, :], ps_nm)
                    musq = small.tile([1, 512], F32, tag="musq")
                    nc.scalar.activation(musq, negmu[:, blk, :], AF.Square)
                    var = small.tile([1, 512], F32, tag="var")
                    nc.vector.tensor_tensor(var, ps_ms, musq, A.subtract)
                    lnv = small.tile([1, 512], F32, tag="lnv")
                    nc.scalar.activation(lnv, var, AF.Ln, bias=eps_t)
                    nc.scalar.activation(rstd[:, blk, :], lnv, AF.Exp, scale=-0.5)
                    if qq_scale:
                        nc.scalar.activation(rstd_s[:, blk, :], lnv, AF.Exp,
                                             scale=-0.5, bias=nlb_t)
                return negmu, rstd, rstd_s

            def rstd_bcast(rstd_row):
                """[1, 512] bf16 -> [128, 512] bf16 (broadcast down partitions)."""
                psb = ps_mm.tile([128, 512], F32, tag="mm")
                mm(psb, ones_row, rstd_row, start=True, stop=True)
                rb = work.tile([128, 512], BF, tag="rb", bufs=2)
                nc.vector.tensor_copy(rb, psb)
                return rb

            def load_w1(i):
                t = work.tile([128, 8, 512], BF, tag="w1", bufs=1)
                nc.sync.dma_start(
                    t, w1_d[i].rearrange("(k p) m -> p k m", p=128))
                return t

            def load_w2(i, col0):
                t = work.tile([128, 4, 1024], BF, tag="w2", bufs=2, name="w2c")
                nc.sync.dma_start(
                    t, w2_d[i][:, col0:col0 + 1024].rearrange(
                        "(k p) m -> p k m", p=128))
                return t

            def dense1(get_tile, n_blocks, w1_sb, blk_idx, negmu, rstd):
                """relu((x-mu)@w1)*rstd -> [128, 4, n_blocks*512] bf16."""
                r = work.tile([128, 4, n_blocks * 512], BF, tag="relu", bufs=1,
                              padded_shape=[128, 4, 1024])
                for blk in range(n_blocks):
                    rb = rstd_bcast(rstd[:, blk, :])
                    for mt in range(4):
                        ps = ps_mm.tile([128, 512], F32, tag="mm")
                        for kt in range(8):
                            mm(ps, w1_sb[:, kt, ts(mt, 128)], get_tile(kt, blk),
                               start=(kt == 0), stop=False)
                        mm(ps, cs_sb[:, blk_idx * 512 + mt * 128:
                                     blk_idx * 512 + (mt + 1) * 128],
                           negmu[:, blk, :], start=False, stop=True)
                        rt = work.tile([128, 512], BF, tag="rt", bufs=2)
                        nc.vector.tensor_scalar_max(rt, ps, 0.0)
                        nc.vector.tensor_tensor(
                            r[:, mt, blk * 512:(blk + 1) * 512], rt, rb, A.mult)
                return r

            # ================= block0 (on q) =================
            qTf = work.tile([128, 8, N], BF, tag="xTf", bufs=1,
                            padded_shape=[128, 8, N])
            nc.sync.dma_start(qTf, qT_d.rearrange("(f p) n -> p f n", p=128))

            negmu_q, rstd_q, _ = stats(
                lambda f, blk: qTf[:, f, blk * 512:(blk + 1) * 512], 2)
            w0_1 = load_w1(0)
            relu0 = dense1(lambda kt, blk: qTf[:, kt, blk * 512:(blk + 1) * 512],
                           2, w0_1, 0, negmu_q, rstd_q)
            # kk: feature-major, act cols [1024:2048)
            w0_2k = load_w2(0, 1024)
            for blk in range(2):
                for ft in range(8):
                    ps = ps_mm.tile([128, 512], F32, tag="mm")
                    for kt in range(4):
                        mm(ps, w0_2k[:, kt, ft * 128:(ft + 1) * 128],
                           relu0[:, kt, blk * 512:(blk + 1) * 512],
                           start=(kt == 0), stop=(kt == 3))
                    nc.vector.tensor_copy(kkT[:, ft, blk * 512:(blk + 1) * 512], ps)
            # vv: token-major, act cols [2048:3072)
            w0_2v = load_w2(0, 2048)
            for jt in range(NT):
                for dh2 in range(2):
                    ps = ps_mm.tile([128, 512], F32, tag="mm")
                    for kt in range(4):
                        mm(ps, relu0[:, kt, jt * 128:(jt + 1) * 128],
                           w0_2v[:, kt, dh2 * 512:(dh2 + 1) * 512],
                           start=(kt == 0), stop=(kt == 3))
                    nc.vector.tensor_copy(vv[:, jt, dh2 * 512:(dh2 + 1) * 512], ps)
            # qq: feature-major, act cols [0:1024), owned tokens, x 1/sqrt(dh)
            # (relu0 was scaled by rstd; qq needs rstd/sqrt(dh) -> extra factor here)
            w0_2q = load_w2(0, 0)
            for ft in range(8):
                ps = ps_mm.tile([128, 512], F32, tag="mm")
                for kt in range(4):
                    mm(ps, w0_2q[:, kt, ft * 128:(ft + 1) * 128],
                       own_ap(relu0, kt), start=(kt == 0), stop=(kt == 3))
                nc.scalar.mul(qqT[:, ft, :], ps, INV_SQRT_DH)

            # dots -> per-head bias deltas (kk at owned rows . pos_delta)
            for h in range(H):
                for k in range(KT):
                    psd = ps_sm.tile([128, 2], F32, tag="sm")
                    mm(psd, kkT[:, h, 256 * k:256 * k + 128],
                       pd_sb[:, 2 * h:2 * h + 2], start=True, stop=True)
                    nc.vector.tensor_copy(delta[:, h, k, :], psd)

            # ================= self-attention =================
            mk1 = work.tile([128, KT, N], BF, tag="mk1", bufs=1)
            mk2 = work.tile([128, KT, N], BF, tag="mk2", bufs=1)
            mk3 = work.tile([128, NT, 128], BF, tag="mk3", bufs=1)
            nc.sync.dma_start(mk1, m1_d.rearrange("(k p) n -> p k n", p=128))
            nc.sync.dma_start(mk2, m2_d.rearrange("(k p) n -> p k n", p=128))
            nc.sync.dma_start(mk3[:, :, :], m3_d[:, :, :])

            for h in range(H):
                dg1 = work.tile([128, KT, 128], BF, tag="dg1", bufs=2)
                dg2 = work.tile([128, KT, 128], BF, tag="dg2", bufs=2)
                for k in range(KT):
                    nc.vector.tensor_scalar_mul(dg1[:, k, :], ident,
                                                delta[:, h, k, 0:1])
                    nc.vector.tensor_scalar_mul(dg2[:, k, :], ident,
                                                delta[:, h, k, 1:2])
                att = work.tile([128, NT, NOWN], BF, tag="att", bufs=2)
                ps_yh = ps_y.tile([128, NOWN], F32, tag="y")
                ps_sum = ps_sm.tile([1, NOWN], F32, tag="sm")
                for jt in range(NT):
                    ilo = KMIN[jt] * 128
                    ps_s = ps_mm.tile([128, NOWN], F32, tag="mm")
                    mm(ps_s[:, ilo:], kkT[:, h, jt * 128:(jt + 1) * 128],
                       qqT[:, h, ilo:], start=True, stop=False)
                    for k in range(KMIN[jt], KT):
                        mm(ps_s[:, k * 128:(k + 1) * 128],
                           mk1[:, k, jt * 128:(jt + 1) * 128], dg1[:, k, :],
                           start=False, stop=False)
                        mm(ps_s[:, k * 128:(k + 1) * 128],
                           mk2[:, k, jt * 128:(jt + 1) * 128], dg2[:, k, :],
                           start=False, stop=False)
                    kd = jt // 2
                    mm(ps_s[:, kd * 128:(kd + 1) * 128],
                       mk3[:, jt, :], diag_kill, start=False, stop=True)
                    nc.scalar.activation(att[:, jt, ilo:], ps_s[:, ilo:], AF.Exp)
                    mm(ps_sum[:, ilo:], ones_col, att[:, jt, ilo:],
                       start=(jt == 0), stop=(jt == NT - 1))
                    mm(ps_yh[:, ilo:], vv[:, jt, h * 128:(h + 1) * 128],
                       att[:, jt, ilo:], start=(jt == 0), stop=(jt == NT - 1))
                lnsum = small.tile([1, NOWN], F32, tag="lnsum")
                nc.scalar.activation(lnsum, ps_sum, AF.Ln)
                rec = small.tile([1, NOWN], BF, tag="rec")
                nc.scalar.activation(rec, lnsum, AF.Exp, scale=-1.0)
                rb = rstd_bcast(rec)
                nc.vector.tensor_tensor(yT[:, h, :], ps_yh, rb, A.mult)

            # ================= block1 + residual =================
            negmu_y, rstd_y, _ = stats(lambda f, blk: yT[:, f, :], 1)
            w1_1 = load_w1(1)
            relu1 = dense1(lambda kt, blk: yT[:, kt, :], 1, w1_1, 1,
                           negmu_y, rstd_y)
            w1_2 = load_w2(1, 0)
            for ft in range(8):
                ps = ps_mm.tile([128, 512], F32, tag="mm")
                for kt in range(4):
                    mm(ps, w1_2[:, kt, ft * 128:(ft + 1) * 128],
                       relu1[:, kt, :], start=(kt == 0), stop=(kt == 3))
                nc.vector.tensor_tensor(
                    q1[:, ft, :].rearrange("p (a c) -> p a c", c=128),
                    ps.rearrange("p (a c) -> p a c", c=128),
                    own_ap(qTf, ft), A.add)

            # ================= block2 -> q2 =================
            negmu_q1, rstd_q1, _ = stats(lambda f, blk: q1[:, f, :], 1)
            w2_1 = load_w1(2)
            relu2 = dense1(lambda kt, blk: q1[:, kt, :], 1, w2_1, 2,
                           negmu_q1, rstd_q1)
            w2_2 = load_w2(2, 0)
            for ft in range(8):
                ps = ps_mm.tile([128, 512], F32, tag="mm")
                for kt in range(4):
                    mm(ps, w2_2[:, kt, ft * 128:(ft + 1) * 128],
                       relu2[:, kt, :], start=(kt == 0), stop=(kt == 3))
                nc.scalar.mul(q2T[:, ft, :], ps, INV_SQRT_DH)

            # ================= block3 (on v_enc) -> k2, v2 =================
            vTf = work.tile([128, 8, N], BF, tag="xTf", bufs=1,
                            padded_shape=[128, 8, N])
            nc.sync.dma_start(vTf, vT_d.rearrange("(f p) n -> p f n", p=128))
            negmu_v, rstd_v, _ = stats(
                lambda f, blk: vTf[:, f, blk * 512:(blk + 1) * 512], 2)
            w3_1 = load_w1(3)
            relu3 = dense1(lambda kt, blk: vTf[:, kt, blk * 512:(blk + 1) * 512],
                           2, w3_1, 3, negmu_v, rstd_v)
            w3_2 = load_w2(3, 0)
            # k2: feature-major, cols [0:1024)
            for blk in range(2):
                for ft in range(8):
                    ps = ps_mm.tile([128, 512], F32, tag="mm")
                    for kt in range(4):
                        mm(ps, w3_2[:, kt, ft * 128:(ft + 1) * 128],
                           relu3[:, kt, blk * 512:(blk + 1) * 512],
                           start=(kt == 0), stop=(kt == 3))
                    nc.vector.tensor_copy(k2T[:, ft, blk * 512:(blk + 1) * 512], ps)
            # v2: token-major, cols [1024:2048)
            w3_2v = load_w2(3, 1024)
            for jt in range(NT):
                for dh2 in range(2):
                    ps = ps_mm.tile([128, 512], F32, tag="mm")
                    for kt in range(4):
                        mm(ps, relu3[:, kt, jt * 128:(jt + 1) * 128],
                           w3_2v[:, kt, dh2 * 512:(dh2 + 1) * 512],
                           start=(kt == 0), stop=(kt == 3))
                    nc.vector.tensor_copy(v2[:, jt, dh2 * 512:(dh2 + 1) * 512], ps)

            # ================= cross-attention =================
            for h in range(H):
                attx = work.tile([128, NT, NOWN], BF, tag="att", bufs=2)
                ps_yh = ps_y.tile([128, NOWN], F32, tag="y")
                ps_sum = ps_sm.tile([1, NOWN], F32, tag="sm")
                for jt in range(NT):
                    ps_s = ps_mm.tile([128, NOWN], F32, tag="mm")
                    mm(ps_s, k2T[:, h, jt * 128:(jt + 1) * 128], q2T[:, h, :],
                       start=True, stop=True)
                    nc.scalar.activation(attx[:, jt, :], ps_s, AF.Exp)
                    mm(ps_sum, ones_col, attx[:, jt, :],
                       start=(jt == 0), stop=(jt == NT - 1))
                    mm(ps_yh, v2[:, jt, h * 128:(h + 1) * 128], attx[:, jt, :],
                       start=(jt == 0), stop=(jt == NT - 1))
                lnsum = small.tile([1, NOWN], F32, tag="lnsum")
                nc.scalar.activation(lnsum, ps_sum, AF.Ln)
                rec = small.tile([1, NOWN], BF, tag="rec")
                nc.scalar.activation(rec, lnsum, AF.Exp, scale=-1.0)
                rb = rstd_bcast(rec)
                nc.vector.tensor_tensor(y2T[:, h, :], ps_yh, rb, A.mult)

            # ================= block4 + residual -> out =================
            negmu_y2, rstd_y2, _ = stats(lambda f, blk: y2T[:, f, :], 1)
            w4_1 = load_w1(4)
            relu4 = dense1(lambda kt, blk: y2T[:, kt, :], 1, w4_1, 4,
                           negmu_y2, rstd_y2)
            w4_2 = load_w2(4, 0)
            for ft in range(8):
                ps = ps_mm.tile([128, 512], F32, tag="mm")
                for kt in range(4):
                    mm(ps, w4_2[:, kt, ft * 128:(ft + 1) * 128],
                       relu4[:, kt, :], start=(kt == 0), stop=(kt == 3))
                out_t = work.tile([128, NOWN], F32, tag="out_t", bufs=2,
                                  name="out_t")
                nc.vector.tensor_tensor(out_t, ps, q1[:, ft, :], A.add)
                nc.sync.dma_start(out_d[ft * 128:(ft + 1) * 128, :], out_t)
    if not nc.is_finalized():
        nc.finalize()
    return nc


_NC = None
TRACE = False
LAST_EXEC_NS = None


def _get_program():
    global _NC
    if _NC is None:
        _NC = build_program()
    return _NC


def _perm_tiles(p):
    return [t for m in range(4) for t in (2 * m + p, 2 * m + 1 - p)]


def _host_prep(inputs):
    """Build the 8 per-core input maps (all host-side, untimed)."""
    f32 = np.float32
    q = np.asarray(inputs["q"], f32)
    v_enc = np.asarray(inputs["v_enc"], f32)
    positions = np.asarray(inputs["positions"])
    pos_table = np.asarray(inputs["pos_table"], f32)

    shared = {}
    for i in range(5):
        w1 = np.asarray(inputs[f"b{i}_w1"], f32)
        w2 = np.asarray(inputs[f"b{i}_w2"], f32)
        shared[f"w{i}_1"] = np.ascontiguousarray(w1).astype(bf16)
        shared[f"w{i}_2"] = np.ascontiguousarray(w2).astype(bf16)
    shared["csum"] = np.concatenate(
        [np.asarray(inputs[f"b{i}_w1"], f32).sum(0) for i in range(5)]
    ).reshape(1, 5 * 512).astype(bf16)
    pt = pos_table.reshape(3, H, DH)
    pd = np.empty((DH, 2 * H), f32)
    for h in range(H):
        pd[:, 2 * h] = pt[1, h] - pt[0, h]
        pd[:, 2 * h + 1] = pt[2, h] - pt[1, h]
    shared["pos_delta"] = pd.astype(bf16)

    in_maps = []
    metas = []
    for c in range(8):
        b, p = c // 2, c % 2
        ptiles = _perm_tiles(p)
        perm = np.concatenate([np.arange(128 * t, 128 * t + 128) for t in ptiles])
        own = perm.reshape(8, 128)[0::2].reshape(-1)   # packed owned rows (orig idx)
        m = dict(shared)
        m["qT"] = np.ascontiguousarray(q[b].T[:, perm]).astype(bf16)
        m["vT"] = np.ascontiguousarray(v_enc[b].T).astype(bf16)
        pos = positions[b]
        po = pos[own][:, perm]
        m["m1"] = (po >= 1).astype(bf16)
        m["m2"] = (po >= 2).astype(bf16)
        m3 = np.zeros((128, NT, 128), f32)
        for jt in range(NT):
            kd = jt // 2
            oi = own[kd * 128:(kd + 1) * 128]
            oj = perm[jt * 128:(jt + 1) * 128]
            m3[:, jt, :] = (oj[None, :] > oi[:, None])
        m["m3"] = m3.astype(bf16)
        in_maps.append(m)
        metas.append((b, own))
    return in_maps, metas


def _install_profile_shim():
    """Provide antenv.axon_hooks (missing in this image) so bass_utils can
    NTFF-profile under axon.  Mirrors trn_boot._ntff_profile_via_ctypes."""
    import sys as _sys
    if "antenv.axon_hooks" in _sys.modules:
        return
    import types
    import ctypes
    import contextlib

    mod = types.ModuleType("antenv.axon_hooks")
    _hook_holder = {"hook": None}

    def set_axon_ntff_profile_hook(h):
        _hook_holder["hook"] = h

    def get_axon_ntff_profile_hook():
        return _hook_holder["hook"]

    mod.set_axon_ntff_profile_hook = set_axon_ntff_profile_hook
    mod.get_axon_ntff_profile_hook = get_axon_ntff_profile_hook
    _sys.modules["antenv.axon_hooks"] = mod

    so_path = "/opt/axon/libaxon_pjrt.so"
    if not os.path.exists(so_path):
        return
    lib = ctypes.CDLL(so_path)
    if not hasattr(lib, "axon_start_nrt_profile"):
        return
    lib.axon_start_nrt_profile.argtypes = [
        ctypes.POINTER(ctypes.c_int64), ctypes.c_size_t]
    lib.axon_start_nrt_profile.restype = ctypes.c_int64
    lib.axon_stop_nrt_profile.argtypes = [ctypes.c_char_p]
    lib.axon_stop_nrt_profile.restype = ctypes.c_int64

    @contextlib.contextmanager
    def _hook(output_dir, device_ids):
        import jax
        jax.devices()
        if device_ids:
            ids = (ctypes.c_int64 * len(device_ids))(*device_ids)
            rc = lib.axon_start_nrt_profile(ids, len(device_ids))
        else:
            rc = lib.axon_start_nrt_profile(None, 0)
        if rc != 0:
            raise RuntimeError(f"axon_start_nrt_profile rc={rc}")
        try:
            yield
        finally:
            n = lib.axon_stop_nrt_profile(str(output_dir).encode())
            print(f"profile: {n} file(s) written to {output_dir}")

    set_axon_ntff_profile_hook(_hook)


def kernel(**inputs):
    global LAST_EXEC_NS
    if TRACE:
        _install_profile_shim()
    from concourse.bass_utils import run_bass_kernel_spmd
    nc = _get_program()
    in_maps, metas = _host_prep(inputs)
    res = run_bass_kernel_spmd(nc, in_maps, core_ids=list(range(8)),
                               trace=TRACE)
    LAST_EXEC_NS = res.exec_time_ns
    out = np.zeros((B, N, D), np.float32)
    for c in range(8):
        b, own = metas[c]
        out[b, own, :] = res.results[c]["out"].T
    return out


if __name__ == "__main__":
    build_program()
    print("build ok")
